# revision 8
# baseline (speedup 1.0000x reference)
"""Expert-parallel MoE MLP (Llama4 text experts) for 8 Trainium2 NeuronCores.

Strategy: core e handles expert e. Tokens are grouped by expert on the host
(indices are sorted, but we argsort for robustness), padded to a common
T_pad, and each core runs a dense gated-MLP over its token block:
    y = (up * silu(gate)) @ W_d,   [gate | up] = x @ W_gu
Everything is computed in the transposed layout (weights are the stationary
matmul operand, tokens stream):  y^T = W_d^T @ (h^T),  h^T = up^T * silu(W_gu^T @ x^T).
bf16 inputs, fp32 PSUM accumulation, fp32 output.
"""

import numpy as np
import ml_dtypes

_BF16 = ml_dtypes.bfloat16
_NC = 8  # cores

_nc_cache: dict = {}
last_run = None  # BassKernelResults of the most recent kernel() call (for test harness)


def _build(T_pad: int, H: int, F: int):
    import concourse.bacc as bacc
    import concourse.mybir as mybir
    from concourse.tile import TileContext

    nc = bacc.Bacc()
    F2 = 2 * F
    xT = nc.dram_tensor("xT", [H, T_pad], mybir.dt.bfloat16, kind="ExternalInput")
    wgu = nc.dram_tensor("wgu", [H, F2], mybir.dt.bfloat16, kind="ExternalInput")
    wd = nc.dram_tensor("wd", [F, H], mybir.dt.bfloat16, kind="ExternalInput")
    yT = nc.dram_tensor("yT", [H, T_pad], mybir.dt.float32, kind="ExternalOutput")

    KB1 = H // 128   # contraction chunks for x @ W_gu
    NF = F2 // 128   # 2F output tiles (first half gate, second half up)
    NG = NF // 2
    KB2 = F // 128   # contraction chunks for h @ W_d
    NH = H // 128    # output tiles of y

    blocks = []
    t0 = 0
    while t0 < T_pad:
        nb = min(512, T_pad - t0)
        blocks.append((t0, nb))
        t0 += nb

    with TileContext(nc) as tc:
        with (
            tc.tile_pool(name="wgu_p", bufs=1) as wgu_p,
            tc.tile_pool(name="wd_p", bufs=1) as wd_p,
            tc.tile_pool(name="x_p", bufs=1) as x_p,
            tc.tile_pool(name="silu_p", bufs=3) as silu_p,
            tc.tile_pool(name="up_p", bufs=3) as up_p,
            tc.tile_pool(name="h_p", bufs=2) as h_p,
            tc.tile_pool(name="y_p", bufs=2) as y_p,
            tc.tile_pool(name="ps1", bufs=4, space="PSUM") as ps1_p,
            tc.tile_pool(name="ps2", bufs=3, space="PSUM") as ps2_p,
        ):
            # x^T first: every matmul needs it.
            x_sb = []
            for k in range(KB1):
                t = x_p.tile([128, T_pad], mybir.dt.bfloat16, name=f"x{k}", tag=f"x{k}")
                nc.sync.dma_start(out=t, in_=xT[k * 128 : (k + 1) * 128, :])
                x_sb.append(t)
            # W_gu: allocate whole-row-chunk tiles but DMA by 512-col groups,
            # f-group-major, so the first f-tiles' weights land first.
            wgu_sb = [
                wgu_p.tile([128, F2], mybir.dt.bfloat16, name=f"wgu{k}", tag=f"wgu{k}")
                for k in range(KB1)
            ]
            for g in range(F2 // 512):
                cs = slice(g * 512, (g + 1) * 512)
                for k in range(KB1):
                    nc.sync.dma_start(
                        out=wgu_sb[k][:, cs], in_=wgu[k * 128 : (k + 1) * 128, cs]
                    )
            wd_sb = []
            for k in range(KB2):
                t = wd_p.tile([128, H], mybir.dt.bfloat16, name=f"wd{k}", tag=f"wd{k}")
                nc.sync.dma_start(out=t, in_=wd[k * 128 : (k + 1) * 128, :])
                wd_sb.append(t)

            for t0, nb in blocks:
                ts = slice(t0, t0 + nb)
                h_tiles = []
                # gate tile i and up tile i+NG paired so the silu tile dies fast
                for i in range(NG):
                    ps_g = ps1_p.tile([128, 512], mybir.dt.float32, tag="ps1")
                    for k in range(KB1):
                        nc.tensor.matmul(
                            out=ps_g[:, :nb],
                            lhsT=wgu_sb[k][:, i * 128 : (i + 1) * 128],
                            rhs=x_sb[k][:, ts],
                            start=(k == 0),
                            stop=(k == KB1 - 1),
                        )
                    st = silu_p.tile([128, 512], mybir.dt.bfloat16, tag="silu")
                    nc.scalar.activation(
                        st[:, :nb], ps_g[:, :nb], mybir.ActivationFunctionType.Silu
                    )
                    ps_u = ps1_p.tile([128, 512], mybir.dt.float32, tag="ps1")
                    iu = i + NG
                    for k in range(KB1):
                        nc.tensor.matmul(
                            out=ps_u[:, :nb],
                            lhsT=wgu_sb[k][:, iu * 128 : (iu + 1) * 128],
                            rhs=x_sb[k][:, ts],
                            start=(k == 0),
                            stop=(k == KB1 - 1),
                        )
                    ut = up_p.tile([128, 512], mybir.dt.bfloat16, tag="up")
                    nc.vector.tensor_copy(ut[:, :nb], ps_u[:, :nb])
                    ht = h_p.tile([128, 512], mybir.dt.bfloat16, tag=f"h{i}")
                    nc.vector.tensor_mul(
                        out=ht[:, :nb], in0=ut[:, :nb], in1=st[:, :nb]
                    )
                    h_tiles.append(ht)
                for hh in range(NH):
                    ps_y = ps2_p.tile([128, 512], mybir.dt.float32, tag="ps2")
                    for k in range(KB2):
                        nc.tensor.matmul(
                            out=ps_y[:, :nb],
                            lhsT=wd_sb[k][:, hh * 128 : (hh + 1) * 128],
                            rhs=h_tiles[k][:, :nb],
                            start=(k == 0),
                            stop=(k == KB2 - 1),
                        )
                    yt = y_p.tile([128, 512], mybir.dt.float32, tag=f"y{hh}")
                    nc.vector.tensor_copy(yt[:, :nb], ps_y[:, :nb])
                    nc.sync.dma_start(
                        out=yT[hh * 128 : (hh + 1) * 128, ts], in_=yt[:, :nb]
                    )
    nc.compile()
    return nc


def kernel(hidden_states, local_expert_indices, gate_up_proj, down_proj):
    from concourse.bass_utils import run_bass_kernel_spmd

    x = np.asarray(hidden_states, dtype=np.float32)
    idx = np.asarray(local_expert_indices).astype(np.int64)
    wgu_all = np.asarray(gate_up_proj, dtype=np.float32)
    wd_all = np.asarray(down_proj, dtype=np.float32)

    T, H = x.shape
    E, _, F2 = wgu_all.shape
    F = F2 // 2
    assert E == _NC

    order = np.argsort(idx, kind="stable")
    counts = np.bincount(idx, minlength=E)
    starts = np.concatenate([[0], np.cumsum(counts)])
    T_pad = max(512, int(-(-counts.max() // 128) * 128))

    key = (T_pad, H, F)
    if key not in _nc_cache:
        _nc_cache[key] = _build(T_pad, H, F)
    nc = _nc_cache[key]

    x_sorted = x[order]
    in_maps = []
    for e in range(E):
        s, c = int(starts[e]), int(counts[e])
        xb = np.zeros((T_pad, H), np.float32)
        xb[:c] = x_sorted[s : s + c]
        in_maps.append(
            {
                "xT": np.ascontiguousarray(xb.T).astype(_BF16),
                "wgu": wgu_all[e].astype(_BF16),
                "wd": wd_all[e].astype(_BF16),
            }
        )

    res = run_bass_kernel_spmd(nc, in_maps, core_ids=list(range(_NC)))
    global last_run
    last_run = res

    out = np.zeros((T, H), np.float32)
    for e in range(E):
        s, c = int(starts[e]), int(counts[e])
        if c:
            out[order[s : s + c]] = res.results[e]["yT"][:, :c].T
    return out
